# revision 10
# baseline (speedup 1.0000x reference)
"""ProbSparse attention Trainium2 kernel (8 NeuronCores, Bass/Tile).

Architecture (B=2, L=2048, d_model=512, H=8, E=64, top-k=38):
  y = OutProj( scatter_softmax_topk( (XWq)(XWk)^T / sqrt(E) ) @ (XWv) )

Sharding: 16 (batch, head) pairs across 8 cores -> core c handles batch c//4,
heads 2*(c%4), 2*(c%4)+1. Each core returns a partial (L, d_model) f32 tile
(its two heads pushed through the output projection); host sums partials.

Numerics: scores are computed in ~fp32 via a 3-term bf16 split packed into
2 matmuls of 128-contraction ([Qh;Ql]@[Kh;Kh] + [Qh;Ql]@[Kl;0]).
Top-38 per row found on the exp'd scores via per-64-chunk top-8 candidates
(DVE max8) + 5 max8/match_replace rounds; threshold = midpoint(v38, v39).
V / attention weights / output projection run in bf16 (fp32 accumulate).
"""

import math
import sys
import types

import numpy as np
import ml_dtypes

B, L, D = 2, 2048, 512
H, E = 8, 64
TOPK = 38
NCORES = 8

_STT_ENGINE = "vector"  # "vector" | "gpsimd"  (engine for the mask pass)
_CACHE = {}


def _bf16(x):
    return x.astype(ml_dtypes.bfloat16)


def _install_patches():
    """Environment workarounds, idempotent (kept for test-harness hooks)."""
    _CACHE["patched"] = True


def _split_multi_waits(nc):
    """This walrus build accepts at most ONE sync wait per instruction.
    Hoist extra waits onto single-wait NoOps inserted just before, on the
    same engine (per-engine program order within a block is preserved)."""
    import concourse.mybir as mybir

    n_split = 0
    for f in nc.m.functions:
        for bb in f.blocks:
            il = bb.instructions
            i = 0
            while i < len(il):
                ins = il[i]
                si = getattr(ins, "sync_info", None)
                if si is not None and len(si.on_wait) > 1:
                    waits = list(si.on_wait)
                    del si.on_wait[:]
                    si.on_wait.append(waits[-1])
                    for k, w in enumerate(waits[:-1]):
                        nop = mybir.InstNoOp(
                            name=f"{ins.name}-wsplit{k}",
                            engine=ins.engine,
                            sync_info=mybir.SyncInfo(
                                on_wait=[w], on_update=[]
                            ),
                            bass_nofuse=True,
                        )
                        il.insert(i, nop)
                        i += 1
                    n_split += 1
                i += 1
    return n_split


def _build_bass():
    """Build the SPMD Bass program (identical on all cores)."""
    import concourse.bass as bass
    import concourse.mybir as mybir
    from concourse.tile import TileContext
    from concourse.masks import make_identity

    fp32 = mybir.dt.float32
    bf = mybir.dt.bfloat16
    Alu = mybir.AluOpType
    Act = mybir.ActivationFunctionType

    nc = bass.Bass()
    xh_d = nc.dram_tensor("xh", (4, 128, L), bf, kind="ExternalInput")
    xl_d = nc.dram_tensor("xl", (4, 128, L), bf, kind="ExternalInput")
    aq_d = nc.dram_tensor("aq", (2, 4, 128, 128), bf, kind="ExternalInput")
    ak_d = nc.dram_tensor("ak", (2, 4, 128, 128), bf, kind="ExternalInput")
    av_d = nc.dram_tensor("av", (4, 128, 128), bf, kind="ExternalInput")
    wo_d = nc.dram_tensor("wo", (128, D), bf, kind="ExternalInput")
    y_d = nc.dram_tensor("y", (L, D), fp32, kind="ExternalOutput")

    with TileContext(nc) as tc:
        stt_eng = nc.gpsimd if _STT_ENGINE == "gpsimd" else nc.vector
        with (
            tc.tile_pool(name="const", bufs=1) as cpool,
            tc.tile_pool(name="persist", bufs=1) as ppool,
        ):
            # ---- load inputs ----
            xh = cpool.tile([128, 4, L], bf)
            xl = cpool.tile([128, 4, L], bf)
            nc.sync.dma_start(xh, xh_d[:].rearrange("c p l -> p c l"))
            nc.sync.dma_start(xl, xl_d[:].rearrange("c p l -> p c l"))
            aq = cpool.tile([128, 2, 4, 128], bf)
            ak = cpool.tile([128, 2, 4, 128], bf)
            nc.sync.dma_start(aq, aq_d[:].rearrange("s c p m -> p s c m"))
            nc.sync.dma_start(ak, ak_d[:].rearrange("s c p m -> p s c m"))
            av = cpool.tile([128, 4, 128], bf)
            nc.sync.dma_start(av, av_d[:].rearrange("c p m -> p c m"))
            wo = cpool.tile([128, D], bf)
            nc.sync.dma_start(wo, wo_d[:])
            ones = cpool.tile([1, 128], fp32)
            nc.vector.memset(ones, 1.0)
            ident = cpool.tile([128, 128], fp32)
            make_identity(nc, ident)

            # ---- persistent per-head tensors ----
            QsT = [ppool.tile([128, L], bf, tag=f"qst{h}", name=f"qst{h}")
                   for h in range(2)]
            Krep = [ppool.tile([128, L], bf, tag=f"krep{h}", name=f"krep{h}")
                    for h in range(2)]
            K2 = [ppool.tile([128, L], bf, tag=f"k2{h}", name=f"k2{h}")
                  for h in range(2)]
            Vsb = ppool.tile([128, 16, 128], bf, tag="v")

            for h in range(2):
                nc.vector.memset(K2[h][64:128, :], 0.0)

            # ---- Q/K projections (3-term bf16 split, fp32 accumulate) ----
            with tc.tile_pool(name="proj_ps", bufs=2, space="PSUM") as proj_ps:
                for which, (asb, hidst, lodst) in enumerate(
                    ((aq, QsT, QsT), (ak, Krep, K2))
                ):
                    for lb in range(4):
                        ls = slice(lb * 512, (lb + 1) * 512)
                        ps = proj_ps.tile([128, 512], fp32, tag="proj")
                        n = 0
                        for t, xs in ((0, xh), (1, xh), (0, xl)):
                            for c in range(4):
                                nc.tensor.matmul(
                                    ps,
                                    asb[:, t, c, :],
                                    xs[:, c, ls],
                                    start=(n == 0),
                                    stop=(n == 11),
                                )
                                n += 1
                        for h in range(2):
                            hs = slice(h * 64, (h + 1) * 64)
                            if which == 0:
                                hi_ap = QsT[h][0:64, ls]
                                lo_ap = QsT[h][64:128, ls]
                            else:
                                hi_ap = Krep[h][0:64, ls]
                                lo_ap = K2[h][0:64, ls]
                            nc.scalar.activation(hi_ap, ps[hs, :], Act.Copy)
                            nc.vector.tensor_tensor(
                                lo_ap, ps[hs, :], hi_ap, op=Alu.subtract
                            )
                            if which == 1:
                                nc.vector.tensor_copy(Krep[h][64:128, ls], hi_ap)

                # ---- V projection (1-term bf16) ----
                for sc in range(16):
                    psv = proj_ps.tile([128, 128], fp32, tag="vproj")
                    for c in range(4):
                        nc.tensor.matmul(
                            psv,
                            xh[:, c, sc * 128:(sc + 1) * 128],
                            av[:, c, :],
                            start=(c == 0),
                            stop=(c == 3),
                        )
                    nc.scalar.activation(Vsb[:, sc, :], psv, Act.Copy)

            # ---- main loop ----
            with (
                tc.tile_pool(name="s_ps", bufs=2, space="PSUM") as s_ps,
                tc.tile_pool(name="av_ps", bufs=2, space="PSUM") as av_ps,
                tc.tile_pool(name="rn_ps", bufs=1, space="PSUM") as rn_ps,
                tc.tile_pool(name="y_ps", bufs=1, space="PSUM") as y_ps,
                tc.tile_pool(name="work", bufs=3) as wpool,
                tc.tile_pool(name="wt", bufs=3) as wtpool,
                tc.tile_pool(name="small", bufs=4) as spool,
            ):
                for lb in range(4):
                    otc = wpool.tile([128, 512], bf, tag="otc")
                    for h in range(2):
                        wt = wtpool.tile([128, 16, 512], bf, tag="wt")
                        R4 = spool.tile([128, 4], fp32, tag="r4")
                        for it in range(4):
                            i = lb * 4 + it
                            qs = QsT[h][:, i * 128:(i + 1) * 128]
                            e32 = wpool.tile([128, L], fp32, tag="e32")
                            for half in range(2):
                                sp = s_ps.tile([128, 1024], fp32, tag="s")
                                for c2 in range(2):
                                    ch = half * 2 + c2
                                    rs = slice(ch * 512, (ch + 1) * 512)
                                    od = sp[:, c2 * 512:(c2 + 1) * 512]
                                    nc.tensor.matmul(
                                        od, qs, Krep[h][:, rs],
                                        start=True, stop=False,
                                    )
                                    nc.tensor.matmul(
                                        od, qs, K2[h][:, rs],
                                        start=False, stop=True,
                                    )
                                nc.scalar.activation(
                                    e32[:, half * 1024:(half + 1) * 1024],
                                    sp, Act.Exp, scale=0.125,
                                )
                            # selection: per-64-chunk top-8 candidates
                            C = wpool.tile([128, 256], fp32, tag="cand")
                            for j in range(32):
                                nc.vector.max(
                                    C[:, j * 8:(j + 1) * 8],
                                    e32[:, j * 64:(j + 1) * 64],
                                )
                            m8 = spool.tile([128, 40], fp32, tag="m8")
                            for r in range(5):
                                nc.vector.max(m8[:, r * 8:(r + 1) * 8], C)
                                if r < 4:
                                    nc.vector.match_replace(
                                        C, m8[:, r * 8:(r + 1) * 8], C, 0.0
                                    )
                            thr = spool.tile([128, 1], fp32, tag="thr")
                            s38 = spool.tile([128, 1], fp32, tag="s38")
                            nc.vector.tensor_add(thr, m8[:, 37:38], m8[:, 38:39])
                            nc.vector.tensor_scalar_mul(thr, thr, 0.5)
                            nc.vector.reduce_sum(
                                s38, m8[:, 0:38], axis=mybir.AxisListType.X
                            )
                            nc.vector.reciprocal(R4[:, it:it + 1], s38)
                            w_u = wpool.tile([128, L], bf, tag="wu")
                            stt_eng.scalar_tensor_tensor(
                                w_u, e32, thr, e32,
                                op0=Alu.is_ge, op1=Alu.mult,
                            )
                            for sc in range(16):
                                nc.sync.dma_start_transpose(
                                    wt[:, sc, it * 128:(it + 1) * 128],
                                    w_u[:, sc * 128:(sc + 1) * 128],
                                )
                        # normalization row: r_row = transpose(R4) as (1, 512)
                        rT = rn_ps.tile([4, 128], fp32, tag="rn")
                        nc.tensor.transpose(rT, R4, ident)
                        rTs = spool.tile([4, 128], fp32, tag="rts")
                        nc.vector.tensor_copy(rTs, rT)
                        rrow = spool.tile([1, 512], fp32, tag="rrow")
                        nc.sync.dma_start(
                            rrow.rearrange("a (b c) -> a b c", b=4), rTs
                        )
                        rrep = rn_ps.tile([64, 512], fp32, tag="rn")
                        nc.tensor.matmul(
                            rrep, ones[:, 0:64], rrow, start=True, stop=True
                        )
                        rrep_sb = spool.tile([64, 512], fp32, tag="rrepsb")
                        nc.vector.tensor_copy(rrep_sb, rrep)
                        out2 = av_ps.tile([64, 512], fp32, tag="av")
                        for sc in range(16):
                            nc.tensor.matmul(
                                out2,
                                Vsb[:, sc, h * 64:(h + 1) * 64],
                                wt[:, sc, :],
                                start=(sc == 0),
                                stop=(sc == 15),
                            )
                        nc.vector.tensor_tensor(
                            otc[h * 64:(h + 1) * 64, :], out2, rrep_sb,
                            op=Alu.mult,
                        )
                    for ls in range(4):
                        yp = y_ps.tile([128, 512], fp32, tag="y")
                        nc.tensor.matmul(
                            yp, otc[:, ls * 128:(ls + 1) * 128], wo,
                            start=True, stop=True,
                        )
                        ysb = wpool.tile([128, 512], fp32, tag="ysb")
                        nc.scalar.activation(ysb, yp, Act.Copy)
                        row0 = lb * 512 + ls * 128
                        nc.sync.dma_start(y_d[row0:row0 + 128, :], ysb)
    _split_multi_waits(nc)
    return nc


def _numpy_reference(x, Wq, bq, Wk, bk, Wv, bv, Wo, bo):
    """Exact numpy replica of the reference (fallback path)."""
    Bb, Ll, d = x.shape
    Hh = H
    Ee = d // Hh
    Q = (x @ Wq.T + bq).reshape(Bb, Ll, Hh, Ee).transpose(0, 2, 1, 3)
    K = (x @ Wk.T + bk).reshape(Bb, Ll, Hh, Ee).transpose(0, 2, 1, 3)
    V = (x @ Wv.T + bv).reshape(Bb, Ll, Hh, Ee).transpose(0, 2, 1, 3)
    scale = Ee ** 0.5
    attn = np.einsum("bhle,bhse->bhls", Q, K) / scale
    k = min(int(5 * math.log(Ll)), Ll)
    idx = np.argsort(-attn, axis=-1, kind="stable")[..., :k]
    topv = np.take_along_axis(attn, idx, axis=-1)
    ex = np.exp(topv - topv.max(-1, keepdims=True))
    sm = ex / ex.sum(-1, keepdims=True)
    attn_w = np.zeros_like(attn)
    np.put_along_axis(attn_w, idx, sm, axis=-1)
    out = np.einsum("bhls,bhse->bhle", attn_w, V)
    out = out.transpose(0, 2, 1, 3).reshape(Bb, Ll, d)
    return (out @ Wo.T + bo).astype(np.float32)


def kernel(**inputs):
    x = np.asarray(inputs["x"], np.float32)
    Wq = np.asarray(inputs["Wq"], np.float32)
    bq = np.asarray(inputs["bq"], np.float32)
    Wk = np.asarray(inputs["Wk"], np.float32)
    bk = np.asarray(inputs["bk"], np.float32)
    Wv = np.asarray(inputs["Wv"], np.float32)
    bv = np.asarray(inputs["bv"], np.float32)
    Wo = np.asarray(inputs["Wo"], np.float32)
    bo = np.asarray(inputs["bo"], np.float32)

    # bq shifts scores per key position and would change the top-k selection;
    # the device kernel assumes it is zero (it always is from setup_inputs).
    # (bk only shifts each query row uniformly - a softmax no-op.)
    if np.any(bq):
        return _numpy_reference(x, Wq, bq, Wk, bk, Wv, bv, Wo, bo)

    _install_patches()
    from concourse.bass_utils import run_bass_kernel_spmd

    if "nc" not in _CACHE:
        _CACHE["nc"] = _build_bass()
    nc = _CACHE["nc"]

    WqT = Wq.T.copy()
    WkT = Wk.T.copy()
    WvT = Wv.T.copy()
    WoT = Wo.T.copy()

    in_maps = []
    for c in range(NCORES):
        b = c // 4
        h0 = 2 * (c % 4)
        he = slice(h0 * E, (h0 + 2) * E)
        xT = np.ascontiguousarray(x[b].T)
        xh = _bf16(xT)
        xlo = _bf16(xT - xh.astype(np.float32))
        Aq = WqT[:, he]
        Ak = WkT[:, he]
        aqh = _bf16(Aq)
        aql = _bf16(Aq - aqh.astype(np.float32))
        akh = _bf16(Ak)
        akl = _bf16(Ak - akh.astype(np.float32))
        in_maps.append({
            "xh": np.ascontiguousarray(xh).reshape(4, 128, L),
            "xl": np.ascontiguousarray(xlo).reshape(4, 128, L),
            "aq": np.ascontiguousarray(
                np.stack([aqh, aql])).reshape(2, 4, 128, 128),
            "ak": np.ascontiguousarray(
                np.stack([akh, akl])).reshape(2, 4, 128, 128),
            "av": np.ascontiguousarray(_bf16(WvT[:, he])).reshape(4, 128, 128),
            "wo": np.ascontiguousarray(_bf16(WoT[he, :])),
        })

    res = run_bass_kernel_spmd(nc, in_maps, core_ids=list(range(NCORES)))

    y = np.zeros((B, L, D), np.float32)
    for c in range(NCORES):
        y[c // 4] += res.results[c]["y"]
    y += (bv @ WoT + bo)[None, None, :]
    return y


if __name__ == "__main__":
    rng = np.random.default_rng(0)
    ins = {
        "x": rng.standard_normal((B, L, D), dtype=np.float32),
        "Wq": rng.standard_normal((D, D), dtype=np.float32) * 0.02,
        "bq": np.zeros(D, np.float32),
        "Wk": rng.standard_normal((D, D), dtype=np.float32) * 0.02,
        "bk": np.zeros(D, np.float32),
        "Wv": rng.standard_normal((D, D), dtype=np.float32) * 0.02,
        "bv": np.zeros(D, np.float32),
        "Wo": rng.standard_normal((D, D), dtype=np.float32) * 0.02,
        "bo": np.zeros(D, np.float32),
    }
    out = kernel(**ins)
    ref = _numpy_reference(**ins)
    err = np.linalg.norm(out - ref) / np.linalg.norm(ref)
    print("self-check rel err:", err)


# revision 12
# speedup vs baseline: 1.3189x; 1.3189x over previous
"""ProbSparse attention Trainium2 kernel (8 NeuronCores, Bass/Tile).

Architecture (B=2, L=2048, d_model=512, H=8, E=64, top-k=38):
  y = OutProj( scatter_softmax_topk( (XWq)(XWk)^T / sqrt(E) ) @ (XWv) )

Sharding: 16 (batch, head) pairs across 8 cores -> core c handles batch c//4,
heads 2*(c%4), 2*(c%4)+1. Each core returns a partial (L, d_model) f32 tile
(its two heads pushed through the output projection); host sums partials.

Numerics / algorithm per (head):
  - Scores in ~fp32 via a 3-term bf16 split packed into 2 matmuls of
    128-contraction: S = [Qh;Ql]^T.T @ ... with replicated/zero layouts.
  - phase 1 (query-major S): top-38 per query row found on raw scores via
    per-64-chunk top-8 candidates (DVE max8) + 5 max8/match_replace rounds;
    threshold thr = midpoint(v38, v39); softmax denom = sum(exp((vi-thr)/8)).
  - phase 2 (key-major S^T, computed by a second PE matmul pass - no
    transposes anywhere): PSUM accumulates u = S^T - thr (rank-1 bf16-split
    update), then w^T = exp(min(BIG*u, u)/8) gives exactly
    exp((S-thr)/8) * [S >= thr]: the top-38 mask applied inside the exp.
  - AV matmul consumes w^T directly; per-query 1/denom applied on the tiny
    (64, 512) output via a rank-1-replicated row; output projection in bf16.
"""

import math

import numpy as np
import ml_dtypes

B, L, D = 2, 2048, 512
H, E = 8, 64
TOPK = 38
NCORES = 8
BIG = 1.0e12
NEG = -1.0e30

_CACHE = {}


def _bf16(x):
    return x.astype(ml_dtypes.bfloat16)


def _install_patches():
    """Kept as a hook point for the test harness; real patching happens in
    _split_multi_waits after trace."""
    _CACHE["patched"] = True


def _split_multi_waits(nc):
    """This walrus build accepts at most ONE sync wait per instruction.
    Hoist extra waits onto single-wait NoOps inserted just before, on the
    same engine (per-engine program order within a block is preserved)."""
    import concourse.mybir as mybir

    n_split = 0
    for f in nc.m.functions:
        for bb in f.blocks:
            il = bb.instructions
            i = 0
            while i < len(il):
                ins = il[i]
                si = getattr(ins, "sync_info", None)
                if si is not None and len(si.on_wait) > 1:
                    waits = list(si.on_wait)
                    del si.on_wait[:]
                    si.on_wait.append(waits[-1])
                    for k, w in enumerate(waits[:-1]):
                        nop = mybir.InstNoOp(
                            name=f"{ins.name}-wsplit{k}",
                            engine=ins.engine,
                            sync_info=mybir.SyncInfo(
                                on_wait=[w], on_update=[]
                            ),
                            bass_nofuse=True,
                        )
                        il.insert(i, nop)
                        i += 1
                    n_split += 1
                i += 1
    return n_split


def _build_bass():
    """Build the SPMD Bass program (identical on all cores)."""
    import concourse.bass as bass
    import concourse.mybir as mybir
    from concourse.tile import TileContext
    from concourse.masks import make_identity

    fp32 = mybir.dt.float32
    bf = mybir.dt.bfloat16
    Alu = mybir.AluOpType
    Act = mybir.ActivationFunctionType
    AxX = mybir.AxisListType.X

    nc = bass.Bass()
    xh_d = nc.dram_tensor("xh", (4, 128, L), bf, kind="ExternalInput")
    xl_d = nc.dram_tensor("xl", (4, 128, L), bf, kind="ExternalInput")
    aq_d = nc.dram_tensor("aq", (2, 4, 128, 128), bf, kind="ExternalInput")
    ak_d = nc.dram_tensor("ak", (2, 4, 128, 128), bf, kind="ExternalInput")
    av_d = nc.dram_tensor("av", (4, 128, 128), bf, kind="ExternalInput")
    wo_d = nc.dram_tensor("wo", (128, D), bf, kind="ExternalInput")
    y_d = nc.dram_tensor("y", (L, D), fp32, kind="ExternalOutput")

    with TileContext(nc) as tc:
        with (
            tc.tile_pool(name="const", bufs=1) as cpool,
            tc.tile_pool(name="persist", bufs=1) as ppool,
        ):
            # ---- constants / weights ----
            aq = cpool.tile([128, 2, 4, 128], bf)
            ak = cpool.tile([128, 2, 4, 128], bf)
            nc.sync.dma_start(aq, aq_d[:].rearrange("s c p m -> p s c m"))
            nc.sync.dma_start(ak, ak_d[:].rearrange("s c p m -> p s c m"))
            av = cpool.tile([128, 4, 128], bf)
            nc.sync.dma_start(av, av_d[:].rearrange("c p m -> p c m"))
            wo = cpool.tile([128, D], bf)
            nc.sync.dma_start(wo, wo_d[:])
            ones_f = cpool.tile([1, 128], fp32)
            nc.vector.memset(ones_f, 1.0)
            onesneg_b = cpool.tile([1, 128], bf)
            nc.vector.memset(onesneg_b, -1.0)
            ident_f = cpool.tile([128, 128], fp32)
            make_identity(nc, ident_f)
            ident_b = cpool.tile([128, 128], bf)
            make_identity(nc, ident_b)

            # ---- persistent per-head operand layouts (bf16 hi/lo stacks) ----
            def per_head(name):
                return [ppool.tile([128, L], bf, tag=f"{name}{h}",
                                   name=f"{name}{h}") for h in range(2)]

            QsT = per_head("qst")    # [Qh; Ql]   (S lhsT)
            Krep = per_head("krep")  # [Kh; Kh]   (S rhs, term 1)
            K2 = per_head("k2")      # [Kl; 0 ]   (S rhs, term 2)
            Ks = per_head("ks")      # [Kh; Kl]   (S^T lhsT)
            Qrep = per_head("qrep")  # [Qh; Qh]   (S^T rhs, term 1)
            Q2 = per_head("q2")      # [Ql; 0 ]   (S^T rhs, term 2)
            Vsb = ppool.tile([128, 16, 128], bf, tag="v")

            for h in range(2):
                nc.vector.memset(K2[h][64:128, :], 0.0)
                nc.vector.memset(Q2[h][64:128, :], 0.0)

            # ---- projections ----
            with (
                tc.tile_pool(name="xin", bufs=1) as xpool,
                tc.tile_pool(name="proj_ps", bufs=2, space="PSUM") as proj_ps,
            ):
                xh = xpool.tile([128, 4, L], bf)
                xl = xpool.tile([128, 4, L], bf)
                nc.sync.dma_start(xh, xh_d[:].rearrange("c p l -> p c l"))
                nc.sync.dma_start(xl, xl_d[:].rearrange("c p l -> p c l"))

                for which, asb in ((0, aq), (1, ak)):
                    for lb in range(4):
                        ls = slice(lb * 512, (lb + 1) * 512)
                        ps = proj_ps.tile([128, 512], fp32, tag="proj")
                        n = 0
                        for t, xs in ((0, xh), (1, xh), (0, xl)):
                            for c in range(4):
                                nc.tensor.matmul(
                                    ps, asb[:, t, c, :], xs[:, c, ls],
                                    start=(n == 0), stop=(n == 11),
                                )
                                n += 1
                        for h in range(2):
                            hs = slice(h * 64, (h + 1) * 64)
                            if which == 0:
                                hi0 = QsT[h][0:64, ls]
                                hi_cps = (Qrep[h][0:64, ls],
                                          Qrep[h][64:128, ls])
                                lo0 = QsT[h][64:128, ls]
                                lo_cps = (Q2[h][0:64, ls],)
                            else:
                                hi0 = Krep[h][0:64, ls]
                                hi_cps = (Krep[h][64:128, ls],
                                          Ks[h][0:64, ls])
                                lo0 = K2[h][0:64, ls]
                                lo_cps = (Ks[h][64:128, ls],)
                            nc.scalar.activation(hi0, ps[hs, :], Act.Copy)
                            for dst in hi_cps:
                                nc.vector.tensor_copy(dst, hi0)
                            nc.vector.tensor_tensor(
                                lo0, ps[hs, :], hi0, op=Alu.subtract
                            )
                            for dst in lo_cps:
                                nc.vector.tensor_copy(dst, lo0)

                # V projection (1-term bf16)
                for sc in range(16):
                    psv = proj_ps.tile([128, 128], fp32, tag="vproj")
                    for c in range(4):
                        nc.tensor.matmul(
                            psv, xh[:, c, sc * 128:(sc + 1) * 128],
                            av[:, c, :], start=(c == 0), stop=(c == 3),
                        )
                    nc.scalar.activation(Vsb[:, sc, :], psv, Act.Copy)

            # ---- main: per head ph1 (selection) + ph2 (masked w^T) + AV ----
            with (
                tc.tile_pool(name="s_ps", bufs=2, space="PSUM") as s_ps,
                tc.tile_pool(name="av_ps", bufs=2, space="PSUM") as av_ps,
                tc.tile_pool(name="rn_ps", bufs=1, space="PSUM") as rn_ps,
                tc.tile_pool(name="y_ps", bufs=1, space="PSUM") as y_ps,
                tc.tile_pool(name="work", bufs=2) as wpool,
                tc.tile_pool(name="wt", bufs=1) as wtpool,
                tc.tile_pool(name="small", bufs=4) as spool,
                tc.tile_pool(name="rows", bufs=1) as rpool,
            ):
                otc = ppool.tile([128, 4, 512], bf, tag="otc")
                for h in range(2):
                    T16 = spool.tile([128, 16], fp32, tag="t16")
                    R16 = spool.tile([128, 16], fp32, tag="r16")
                    # ---------------- phase 1: selection ----------------
                    for i in range(16):
                        qs = QsT[h][:, i * 128:(i + 1) * 128]
                        ssb = wpool.tile([128, L], fp32, tag="big32")
                        for half in range(2):
                            sp = s_ps.tile([128, 1024], fp32, tag="s")
                            for c2 in range(2):
                                ch = half * 2 + c2
                                rs = slice(ch * 512, (ch + 1) * 512)
                                od = sp[:, c2 * 512:(c2 + 1) * 512]
                                nc.tensor.matmul(od, qs, Krep[h][:, rs],
                                                 start=True, stop=False)
                                nc.tensor.matmul(od, qs, K2[h][:, rs],
                                                 start=False, stop=True)
                            nc.scalar.activation(
                                ssb[:, half * 1024:(half + 1) * 1024],
                                sp, Act.Copy,
                            )
                        C = wpool.tile([128, 256], fp32, tag="cand")
                        for j in range(32):
                            nc.vector.max(C[:, j * 8:(j + 1) * 8],
                                          ssb[:, j * 64:(j + 1) * 64])
                        m8 = spool.tile([128, 40], fp32, tag="m8")
                        for r in range(5):
                            nc.vector.max(m8[:, r * 8:(r + 1) * 8], C)
                            if r < 4:
                                nc.vector.match_replace(
                                    C, m8[:, r * 8:(r + 1) * 8], C, NEG)
                        thr = T16[:, i:i + 1]
                        nc.vector.tensor_add(thr, m8[:, 37:38], m8[:, 38:39])
                        nc.vector.tensor_scalar_mul(thr, thr, 0.5)
                        thrn = spool.tile([128, 1], fp32, tag="thrn")
                        nc.vector.tensor_scalar_mul(thrn, thr, -0.125)
                        e38 = spool.tile([128, 38], fp32, tag="e38")
                        nc.scalar.activation(e38, m8[:, 0:38], Act.Exp,
                                             bias=thrn, scale=0.125)
                        s38 = spool.tile([128, 1], fp32, tag="s38")
                        nc.vector.reduce_sum(s38, e38, axis=AxX)
                        nc.vector.reciprocal(R16[:, i:i + 1], s38)

                    # ---- head tail: thr/r rows via PE transpose ----
                    T16h = spool.tile([128, 16], bf, tag="t16h")
                    T16l = spool.tile([128, 16], bf, tag="t16l")
                    nc.vector.tensor_copy(T16h, T16)
                    nc.vector.tensor_tensor(T16l, T16, T16h, op=Alu.subtract)
                    throwh = rpool.tile([1, 2048], bf, tag="throwh")
                    throwl = rpool.tile([1, 2048], bf, tag="throwl")
                    rrow = rpool.tile([1, 2048], fp32, tag="rrow")
                    for src, idn, dst in (
                        (T16h, ident_b, throwh),
                        (T16l, ident_b, throwl),
                        (R16, ident_f, rrow),
                    ):
                        tp = rn_ps.tile([16, 128], src.dtype, tag="rn",
                                        name="tp")
                        nc.tensor.transpose(tp, src, idn)
                        tps = spool.tile([16, 128], src.dtype, tag="tps")
                        nc.vector.tensor_copy(tps, tp)
                        nc.sync.dma_start(
                            dst.rearrange("a (b c) -> a b c", b=16), tps)
                    rrep = rpool.tile([64, 2048], fp32, tag="rrep")
                    for lb in range(4):
                        rp = rn_ps.tile([64, 512], fp32, tag="rn", name="rp")
                        nc.tensor.matmul(
                            rp, ones_f[:, 0:64],
                            rrow[:, lb * 512:(lb + 1) * 512],
                            start=True, stop=True,
                        )
                        nc.scalar.activation(
                            rrep[:, lb * 512:(lb + 1) * 512], rp, Act.Copy)

                    # ------------- phase 2: w^T via S^T - thr -------------
                    wt = wtpool.tile([128, 16, L], bf, tag="wt")
                    for sc in range(16):
                        ks = Ks[h][:, sc * 128:(sc + 1) * 128]
                        usb = wpool.tile([128, L], fp32, tag="big32",
                                         name="usb")
                        for half in range(2):
                            up = s_ps.tile([128, 1024], fp32, tag="s",
                                           name="up")
                            for c2 in range(2):
                                ch = half * 2 + c2
                                rs = slice(ch * 512, (ch + 1) * 512)
                                od = up[:, c2 * 512:(c2 + 1) * 512]
                                nc.tensor.matmul(od, ks, Qrep[h][:, rs],
                                                 start=True, stop=False)
                                nc.tensor.matmul(od, ks, Q2[h][:, rs],
                                                 start=False, stop=False)
                                nc.tensor.matmul(od, onesneg_b, throwh[:, rs],
                                                 start=False, stop=False)
                                nc.tensor.matmul(od, onesneg_b, throwl[:, rs],
                                                 start=False, stop=True)
                            nc.scalar.activation(
                                usb[:, half * 1024:(half + 1) * 1024],
                                up, Act.Copy,
                            )
                        # z = min(BIG*u, u): u for selected, -huge otherwise
                        nc.vector.scalar_tensor_tensor(
                            usb, usb, BIG, usb, op0=Alu.mult, op1=Alu.min)
                        nc.scalar.activation(wt[:, sc, :], usb, Act.Exp,
                                             scale=0.125)

                    # ---------------- AV + normalize ----------------
                    for lb in range(4):
                        lsl = slice(lb * 512, (lb + 1) * 512)
                        out2 = av_ps.tile([64, 512], fp32, tag="av")
                        for sc in range(16):
                            nc.tensor.matmul(
                                out2, Vsb[:, sc, h * 64:(h + 1) * 64],
                                wt[:, sc, lsl],
                                start=(sc == 0), stop=(sc == 15),
                            )
                        nc.vector.tensor_tensor(
                            otc[h * 64:(h + 1) * 64, lb, :], out2,
                            rrep[:, lsl], op=Alu.mult,
                        )

                # ---------------- output projection ----------------
                for lb in range(4):
                    for ls in range(4):
                        yp = y_ps.tile([128, 512], fp32, tag="y")
                        nc.tensor.matmul(
                            yp, otc[:, lb, ls * 128:(ls + 1) * 128], wo,
                            start=True, stop=True,
                        )
                        ysb = wpool.tile([128, 512], fp32, tag="ysb")
                        nc.scalar.activation(ysb, yp, Act.Copy)
                        row0 = lb * 512 + ls * 128
                        nc.sync.dma_start(y_d[row0:row0 + 128, :], ysb)
    _split_multi_waits(nc)
    return nc


def _numpy_reference(x, Wq, bq, Wk, bk, Wv, bv, Wo, bo):
    """Exact numpy replica of the reference (fallback path)."""
    Bb, Ll, d = x.shape
    Hh = H
    Ee = d // Hh
    Q = (x @ Wq.T + bq).reshape(Bb, Ll, Hh, Ee).transpose(0, 2, 1, 3)
    K = (x @ Wk.T + bk).reshape(Bb, Ll, Hh, Ee).transpose(0, 2, 1, 3)
    V = (x @ Wv.T + bv).reshape(Bb, Ll, Hh, Ee).transpose(0, 2, 1, 3)
    scale = Ee ** 0.5
    attn = np.einsum("bhle,bhse->bhls", Q, K) / scale
    k = min(int(5 * math.log(Ll)), Ll)
    idx = np.argsort(-attn, axis=-1, kind="stable")[..., :k]
    topv = np.take_along_axis(attn, idx, axis=-1)
    ex = np.exp(topv - topv.max(-1, keepdims=True))
    sm = ex / ex.sum(-1, keepdims=True)
    attn_w = np.zeros_like(attn)
    np.put_along_axis(attn_w, idx, sm, axis=-1)
    out = np.einsum("bhls,bhse->bhle", attn_w, V)
    out = out.transpose(0, 2, 1, 3).reshape(Bb, Ll, d)
    return (out @ Wo.T + bo).astype(np.float32)


def kernel(**inputs):
    x = np.asarray(inputs["x"], np.float32)
    Wq = np.asarray(inputs["Wq"], np.float32)
    bq = np.asarray(inputs["bq"], np.float32)
    Wk = np.asarray(inputs["Wk"], np.float32)
    bk = np.asarray(inputs["bk"], np.float32)
    Wv = np.asarray(inputs["Wv"], np.float32)
    bv = np.asarray(inputs["bv"], np.float32)
    Wo = np.asarray(inputs["Wo"], np.float32)
    bo = np.asarray(inputs["bo"], np.float32)

    # bq shifts scores per key position and would change the top-k selection;
    # the device kernel assumes it is zero (it always is from setup_inputs).
    # (bk only shifts each query row uniformly - a softmax no-op. bv and bo
    # are applied exactly on the host below.)
    if np.any(bq):
        return _numpy_reference(x, Wq, bq, Wk, bk, Wv, bv, Wo, bo)

    _install_patches()
    from concourse.bass_utils import run_bass_kernel_spmd

    if "nc" not in _CACHE:
        _CACHE["nc"] = _build_bass()
    nc = _CACHE["nc"]

    WqT = Wq.T.copy()
    WkT = Wk.T.copy()
    WvT = Wv.T.copy()
    WoT = Wo.T.copy()

    in_maps = []
    for c in range(NCORES):
        b = c // 4
        h0 = 2 * (c % 4)
        he = slice(h0 * E, (h0 + 2) * E)
        xT = np.ascontiguousarray(x[b].T)
        xh = _bf16(xT)
        xlo = _bf16(xT - xh.astype(np.float32))
        Aq = WqT[:, he]
        Ak = WkT[:, he]
        aqh = _bf16(Aq)
        aql = _bf16(Aq - aqh.astype(np.float32))
        akh = _bf16(Ak)
        akl = _bf16(Ak - akh.astype(np.float32))
        in_maps.append({
            "xh": np.ascontiguousarray(xh).reshape(4, 128, L),
            "xl": np.ascontiguousarray(xlo).reshape(4, 128, L),
            "aq": np.ascontiguousarray(
                np.stack([aqh, aql])).reshape(2, 4, 128, 128),
            "ak": np.ascontiguousarray(
                np.stack([akh, akl])).reshape(2, 4, 128, 128),
            "av": np.ascontiguousarray(_bf16(WvT[:, he])).reshape(4, 128, 128),
            "wo": np.ascontiguousarray(_bf16(WoT[he, :])),
        })

    res = run_bass_kernel_spmd(nc, in_maps, core_ids=list(range(NCORES)))

    y = np.zeros((B, L, D), np.float32)
    for c in range(NCORES):
        y[c // 4] += res.results[c]["y"]
    y += (bv @ WoT + bo)[None, None, :]
    return y


if __name__ == "__main__":
    rng = np.random.default_rng(0)
    ins = {
        "x": rng.standard_normal((B, L, D), dtype=np.float32),
        "Wq": rng.standard_normal((D, D), dtype=np.float32) * 0.02,
        "bq": np.zeros(D, np.float32),
        "Wk": rng.standard_normal((D, D), dtype=np.float32) * 0.02,
        "bk": np.zeros(D, np.float32),
        "Wv": rng.standard_normal((D, D), dtype=np.float32) * 0.02,
        "bv": np.zeros(D, np.float32),
        "Wo": rng.standard_normal((D, D), dtype=np.float32) * 0.02,
        "bo": np.zeros(D, np.float32),
    }
    out = kernel(**ins)
    ref = _numpy_reference(**ins)
    err = np.linalg.norm(out - ref) / np.linalg.norm(ref)
    print("self-check rel err:", err)


# revision 14
# speedup vs baseline: 1.6520x; 1.2526x over previous
"""ProbSparse attention Trainium2 kernel (8 NeuronCores, Bass/Tile).

Architecture (B=2, L=2048, d_model=512, H=8, E=64, top-k=38):
  y = OutProj( scatter_softmax_topk( (XWq)(XWk)^T / sqrt(E) ) @ (XWv) )

Sharding: 16 (batch, head) pairs across 8 cores -> core c handles batch c//4,
heads 2*(c%4), 2*(c%4)+1. Each core returns a partial (L, d_model) f32 tile
(its two heads pushed through the output projection); host sums partials.

Numerics / algorithm per (head):
  - Scores in ~fp32 via a 3-term bf16 split packed into 2 matmuls of
    128-contraction: S = [Qh;Ql]^T.T @ ... with replicated/zero layouts.
  - phase 1 (query-major S): top-38 per query row found on raw scores via
    per-64-chunk top-8 candidates (DVE max8) + 5 max8/match_replace rounds;
    threshold thr = midpoint(v38, v39); softmax denom = sum(exp((vi-thr)/8)).
  - phase 2 (key-major S^T, computed by a second PE matmul pass - no
    transposes anywhere): PSUM accumulates u = S^T - thr (rank-1 bf16-split
    update), then w^T = exp(min(BIG*u, u)/8) gives exactly
    exp((S-thr)/8) * [S >= thr]: the top-38 mask applied inside the exp.
  - AV matmul consumes w^T directly; per-query 1/denom applied on the tiny
    (64, 512) output via a rank-1-replicated row; output projection in bf16.
"""

import math

import numpy as np
import ml_dtypes

B, L, D = 2, 2048, 512
H, E = 8, 64
TOPK = 38
NCORES = 8
BIG = 1.0e12
NEG = -1.0e30

_CACHE = {}


def _bf16(x):
    return x.astype(ml_dtypes.bfloat16)


def _install_patches():
    """Kept as a hook point for the test harness; real patching happens in
    _split_multi_waits after trace."""
    _CACHE["patched"] = True


def _split_multi_waits(nc):
    """This walrus build accepts at most ONE sync wait per instruction.
    Hoist extra waits onto single-wait NoOps inserted just before, on the
    same engine (per-engine program order within a block is preserved)."""
    import concourse.mybir as mybir

    n_split = 0
    for f in nc.m.functions:
        for bb in f.blocks:
            il = bb.instructions
            i = 0
            while i < len(il):
                ins = il[i]
                si = getattr(ins, "sync_info", None)
                if si is not None and len(si.on_wait) > 1:
                    waits = list(si.on_wait)
                    del si.on_wait[:]
                    si.on_wait.append(waits[-1])
                    for k, w in enumerate(waits[:-1]):
                        nop = mybir.InstNoOp(
                            name=f"{ins.name}-wsplit{k}",
                            engine=ins.engine,
                            sync_info=mybir.SyncInfo(
                                on_wait=[w], on_update=[]
                            ),
                            bass_nofuse=True,
                        )
                        il.insert(i, nop)
                        i += 1
                    n_split += 1
                i += 1
    return n_split


def _build_bass():
    """Build the SPMD Bass program (identical on all cores)."""
    import concourse.bass as bass
    import concourse.mybir as mybir
    from concourse.tile import TileContext
    from concourse.masks import make_identity

    fp32 = mybir.dt.float32
    bf = mybir.dt.bfloat16
    Alu = mybir.AluOpType
    Act = mybir.ActivationFunctionType
    AxX = mybir.AxisListType.X

    nc = bass.Bass()
    xh_d = nc.dram_tensor("xh", (4, 128, L), bf, kind="ExternalInput")
    xl_d = nc.dram_tensor("xl", (4, 128, L), bf, kind="ExternalInput")
    aq_d = nc.dram_tensor("aq", (2, 4, 128, 128), bf, kind="ExternalInput")
    ak_d = nc.dram_tensor("ak", (2, 4, 128, 128), bf, kind="ExternalInput")
    av_d = nc.dram_tensor("av", (4, 128, 128), bf, kind="ExternalInput")
    wo_d = nc.dram_tensor("wo", (128, D), bf, kind="ExternalInput")
    y_d = nc.dram_tensor("y", (L, D), fp32, kind="ExternalOutput")

    with TileContext(nc) as tc:
        with (
            tc.tile_pool(name="const", bufs=1) as cpool,
            tc.tile_pool(name="persist", bufs=1) as ppool,
        ):
            # ---- constants / weights ----
            aq = cpool.tile([128, 2, 4, 128], bf)
            ak = cpool.tile([128, 2, 4, 128], bf)
            nc.sync.dma_start(aq, aq_d[:].rearrange("s c p m -> p s c m"))
            nc.sync.dma_start(ak, ak_d[:].rearrange("s c p m -> p s c m"))
            av = cpool.tile([128, 4, 128], bf)
            nc.sync.dma_start(av, av_d[:].rearrange("c p m -> p c m"))
            wo = cpool.tile([128, D], bf)
            nc.sync.dma_start(wo, wo_d[:])
            ones_f = cpool.tile([1, 128], fp32)
            nc.vector.memset(ones_f, 1.0)
            onesneg2 = cpool.tile([2, 128], bf)
            nc.vector.memset(onesneg2, -1.0)
            ident_f = cpool.tile([128, 128], fp32)
            make_identity(nc, ident_f)
            ident_b = cpool.tile([128, 128], bf)
            make_identity(nc, ident_b)

            # ---- persistent per-head operand layouts (bf16 hi/lo stacks) ----
            def per_head(name):
                return [ppool.tile([128, L], bf, tag=f"{name}{h}",
                                   name=f"{name}{h}") for h in range(2)]

            QsT = per_head("qst")    # [Qh; Ql]   (S lhsT)
            Krep = per_head("krep")  # [Kh; Kh]   (S rhs, term 1)
            K2 = per_head("k2")      # [Kl; 0 ]   (S rhs, term 2)
            Ks = per_head("ks")      # [Kh; Kl]   (S^T lhsT)
            Qrep = per_head("qrep")  # [Qh; Qh]   (S^T rhs, term 1)
            Q2 = per_head("q2")      # [Ql; 0 ]   (S^T rhs, term 2)
            Vsb = ppool.tile([128, 16, 128], bf, tag="v")

            for h in range(2):
                nc.vector.memset(K2[h][64:128, :], 0.0)
                nc.vector.memset(Q2[h][64:128, :], 0.0)

            # ---- projections ----
            with (
                tc.tile_pool(name="xin", bufs=1) as xpool,
                tc.tile_pool(name="proj_ps", bufs=2, space="PSUM") as proj_ps,
            ):
                xh = xpool.tile([128, 4, L], bf)
                xl = xpool.tile([128, 4, L], bf)
                nc.sync.dma_start(xh, xh_d[:].rearrange("c p l -> p c l"))
                nc.sync.dma_start(xl, xl_d[:].rearrange("c p l -> p c l"))

                for which, asb in ((0, aq), (1, ak)):
                    for lb in range(4):
                        ls = slice(lb * 512, (lb + 1) * 512)
                        ps = proj_ps.tile([128, 512], fp32, tag="proj")
                        n = 0
                        for t, xs in ((0, xh), (1, xh), (0, xl)):
                            for c in range(4):
                                nc.tensor.matmul(
                                    ps, asb[:, t, c, :], xs[:, c, ls],
                                    start=(n == 0), stop=(n == 11),
                                )
                                n += 1
                        for h in range(2):
                            hs = slice(h * 64, (h + 1) * 64)
                            if which == 0:
                                hi0 = QsT[h][0:64, ls]
                                hi_cps = (Qrep[h][0:64, ls],
                                          Qrep[h][64:128, ls])
                                lo0 = QsT[h][64:128, ls]
                                lo_cps = (Q2[h][0:64, ls],)
                            else:
                                hi0 = Krep[h][0:64, ls]
                                hi_cps = (Krep[h][64:128, ls],
                                          Ks[h][0:64, ls])
                                lo0 = K2[h][0:64, ls]
                                lo_cps = (Ks[h][64:128, ls],)
                            nc.scalar.activation(hi0, ps[hs, :], Act.Copy)
                            for dst in hi_cps:
                                nc.vector.tensor_copy(dst, hi0)
                            nc.vector.tensor_tensor(
                                lo0, ps[hs, :], hi0, op=Alu.subtract
                            )
                            for dst in lo_cps:
                                nc.vector.tensor_copy(dst, lo0)

                # V projection (1-term bf16)
                for sc in range(16):
                    psv = proj_ps.tile([128, 128], fp32, tag="vproj")
                    for c in range(4):
                        nc.tensor.matmul(
                            psv, xh[:, c, sc * 128:(sc + 1) * 128],
                            av[:, c, :], start=(c == 0), stop=(c == 3),
                        )
                    nc.scalar.activation(Vsb[:, sc, :], psv, Act.Copy)

            # ---- main: per head ph1 (selection) + ph2 (masked w^T) + AV ----
            with (
                tc.tile_pool(name="s_ps", bufs=2, space="PSUM") as s_ps,
                tc.tile_pool(name="av_ps", bufs=2, space="PSUM") as av_ps,
                tc.tile_pool(name="rn_ps", bufs=1, space="PSUM") as rn_ps,
                tc.tile_pool(name="y_ps", bufs=1, space="PSUM") as y_ps,
                tc.tile_pool(name="work", bufs=2) as wpool,
                tc.tile_pool(name="wt", bufs=1) as wtpool,
                tc.tile_pool(name="small", bufs=4) as spool,
                tc.tile_pool(name="rows", bufs=1) as rpool,
            ):
                otc = ppool.tile([128, 4, 512], bf, tag="otc")
                T16s, R16s, thr2s, rrows, rreps, wts = {}, {}, {}, {}, {}, {}

                def ph1_tile(h, i):
                    qs = QsT[h][:, i * 128:(i + 1) * 128]
                    ssb = wpool.tile([128, L], fp32, tag="big32", name="ssb")
                    for half in range(2):
                        sp = s_ps.tile([128, 1024], fp32, tag="s", name="sp")
                        for c2 in range(2):
                            ch = half * 2 + c2
                            rs = slice(ch * 512, (ch + 1) * 512)
                            od = sp[:, c2 * 512:(c2 + 1) * 512]
                            nc.tensor.matmul(od, qs, Krep[h][:, rs],
                                             start=True, stop=False)
                            nc.tensor.matmul(od, qs, K2[h][:, rs],
                                             start=False, stop=True)
                        nc.scalar.activation(
                            ssb[:, half * 1024:(half + 1) * 1024],
                            sp, Act.Copy,
                        )
                    C = wpool.tile([128, 256], fp32, tag="cand", name="C")
                    for j in range(32):
                        nc.vector.max(C[:, j * 8:(j + 1) * 8],
                                      ssb[:, j * 64:(j + 1) * 64])
                    m8 = spool.tile([128, 40], fp32, tag="m8", name="m8")
                    for r in range(5):
                        nc.vector.max(m8[:, r * 8:(r + 1) * 8], C)
                        if r < 4:
                            nc.vector.match_replace(
                                C, m8[:, r * 8:(r + 1) * 8], C, NEG)
                    thr = T16s[h][:, i:i + 1]
                    nc.vector.tensor_add(thr, m8[:, 37:38], m8[:, 38:39])
                    nc.vector.tensor_scalar_mul(thr, thr, 0.5)
                    thrn = spool.tile([128, 1], fp32, tag="thrn", name="thrn")
                    nc.vector.tensor_scalar_mul(thrn, thr, -0.125)
                    e38 = spool.tile([128, 38], fp32, tag="e38", name="e38")
                    nc.scalar.activation(e38, m8[:, 0:38], Act.Exp,
                                         bias=thrn, scale=0.125)
                    s38 = spool.tile([128, 1], fp32, tag="s38", name="s38")
                    nc.vector.reduce_sum(s38, e38, axis=AxX)
                    nc.vector.reciprocal(R16s[h][:, i:i + 1], s38)

                def head_tail(h):
                    T16, R16 = T16s[h], R16s[h]
                    T16h = spool.tile([128, 16], bf, tag="t16h", name="t16h")
                    T16l = spool.tile([128, 16], bf, tag="t16l", name="t16l")
                    nc.vector.tensor_copy(T16h, T16)
                    nc.vector.tensor_tensor(T16l, T16, T16h, op=Alu.subtract)
                    thr2 = rpool.tile([2, 2048], bf, tag="thr2", name="thr2")
                    rrow = rpool.tile([1, 2048], fp32, tag="rrow", name="rrow")
                    for tsrc, idn, dst in (
                        (T16h, ident_b, thr2[0:1, :]),
                        (T16l, ident_b, thr2[1:2, :]),
                        (R16, ident_f, rrow),
                    ):
                        tp = rn_ps.tile([16, 128], tsrc.dtype, tag="rn",
                                        name="tp")
                        nc.tensor.transpose(tp, tsrc, idn)
                        tps = spool.tile([16, 128], tsrc.dtype, tag="tps",
                                         name="tps")
                        nc.vector.tensor_copy(tps, tp)
                        nc.sync.dma_start(
                            dst.rearrange("a (b c) -> a b c", b=16), tps)
                    rrep = rpool.tile([64, 2048], fp32, tag="rrep",
                                      name="rrep")
                    for lb in range(4):
                        rp = rn_ps.tile([64, 512], fp32, tag="rn", name="rp")
                        nc.tensor.matmul(
                            rp, ones_f[:, 0:64],
                            rrow[:, lb * 512:(lb + 1) * 512],
                            start=True, stop=True,
                        )
                        nc.scalar.activation(
                            rrep[:, lb * 512:(lb + 1) * 512], rp, Act.Copy)
                    thr2s[h], rrows[h], rreps[h] = thr2, rrow, rrep

                def ph2_tile(h, sc):
                    ks = Ks[h][:, sc * 128:(sc + 1) * 128]
                    thr2 = thr2s[h]
                    wt = wts[h]
                    usb = wpool.tile([128, L], fp32, tag="big32", name="usb")
                    for half in range(2):
                        up = s_ps.tile([128, 1024], fp32, tag="s", name="up")
                        for c2 in range(2):
                            ch = half * 2 + c2
                            rs = slice(ch * 512, (ch + 1) * 512)
                            od = up[:, c2 * 512:(c2 + 1) * 512]
                            nc.tensor.matmul(od, ks, Qrep[h][:, rs],
                                             start=True, stop=False)
                            nc.tensor.matmul(od, ks, Q2[h][:, rs],
                                             start=False, stop=False)
                            nc.tensor.matmul(od, onesneg2, thr2[:, rs],
                                             start=False, stop=True)
                        nc.scalar.activation(
                            usb[:, half * 1024:(half + 1) * 1024],
                            up, Act.Copy,
                        )
                    # z = min(BIG*u, u): u for selected, -huge otherwise
                    nc.vector.scalar_tensor_tensor(
                        usb, usb, BIG, usb, op0=Alu.mult, op1=Alu.min)
                    nc.scalar.activation(wt[:, sc, :], usb, Act.Exp,
                                         scale=0.125)

                def av_head(h):
                    wt, rrep = wts[h], rreps[h]
                    for lb in range(4):
                        lsl = slice(lb * 512, (lb + 1) * 512)
                        out2 = av_ps.tile([64, 512], fp32, tag="av",
                                          name="out2")
                        for sc in range(16):
                            nc.tensor.matmul(
                                out2, Vsb[:, sc, h * 64:(h + 1) * 64],
                                wt[:, sc, lsl],
                                start=(sc == 0), stop=(sc == 15),
                            )
                        nc.vector.tensor_tensor(
                            otc[h * 64:(h + 1) * 64, lb, :], out2,
                            rrep[:, lsl], op=Alu.mult,
                        )

                for h in range(2):
                    T16s[h] = spool.tile([128, 16], fp32, tag=f"t16_{h}",
                                         name=f"t16_{h}")
                    R16s[h] = spool.tile([128, 16], fp32, tag=f"r16_{h}",
                                         name=f"r16_{h}")

                # interleave: ph1(h0); tail(h0); [ph2(h0, k) | ph1(h1, k)];
                # av(h0); tail(h1); ph2(h1); av(h1)
                for i in range(16):
                    ph1_tile(0, i)
                head_tail(0)
                wts[0] = wtpool.tile([128, 16, L], bf, tag="wt", name="wt0")
                for k in range(16):
                    ph2_tile(0, k)
                    ph1_tile(1, k)
                av_head(0)
                head_tail(1)
                wts[1] = wtpool.tile([128, 16, L], bf, tag="wt", name="wt1")
                for k in range(16):
                    ph2_tile(1, k)
                av_head(1)

                # ---------------- output projection ----------------
                for lb in range(4):
                    for ls in range(4):
                        yp = y_ps.tile([128, 512], fp32, tag="y")
                        nc.tensor.matmul(
                            yp, otc[:, lb, ls * 128:(ls + 1) * 128], wo,
                            start=True, stop=True,
                        )
                        ysb = wpool.tile([128, 512], fp32, tag="ysb")
                        nc.scalar.activation(ysb, yp, Act.Copy)
                        row0 = lb * 512 + ls * 128
                        nc.sync.dma_start(y_d[row0:row0 + 128, :], ysb)
    _split_multi_waits(nc)
    return nc


def _numpy_reference(x, Wq, bq, Wk, bk, Wv, bv, Wo, bo):
    """Exact numpy replica of the reference (fallback path)."""
    Bb, Ll, d = x.shape
    Hh = H
    Ee = d // Hh
    Q = (x @ Wq.T + bq).reshape(Bb, Ll, Hh, Ee).transpose(0, 2, 1, 3)
    K = (x @ Wk.T + bk).reshape(Bb, Ll, Hh, Ee).transpose(0, 2, 1, 3)
    V = (x @ Wv.T + bv).reshape(Bb, Ll, Hh, Ee).transpose(0, 2, 1, 3)
    scale = Ee ** 0.5
    attn = np.einsum("bhle,bhse->bhls", Q, K) / scale
    k = min(int(5 * math.log(Ll)), Ll)
    idx = np.argsort(-attn, axis=-1, kind="stable")[..., :k]
    topv = np.take_along_axis(attn, idx, axis=-1)
    ex = np.exp(topv - topv.max(-1, keepdims=True))
    sm = ex / ex.sum(-1, keepdims=True)
    attn_w = np.zeros_like(attn)
    np.put_along_axis(attn_w, idx, sm, axis=-1)
    out = np.einsum("bhls,bhse->bhle", attn_w, V)
    out = out.transpose(0, 2, 1, 3).reshape(Bb, Ll, d)
    return (out @ Wo.T + bo).astype(np.float32)


def kernel(**inputs):
    x = np.asarray(inputs["x"], np.float32)
    Wq = np.asarray(inputs["Wq"], np.float32)
    bq = np.asarray(inputs["bq"], np.float32)
    Wk = np.asarray(inputs["Wk"], np.float32)
    bk = np.asarray(inputs["bk"], np.float32)
    Wv = np.asarray(inputs["Wv"], np.float32)
    bv = np.asarray(inputs["bv"], np.float32)
    Wo = np.asarray(inputs["Wo"], np.float32)
    bo = np.asarray(inputs["bo"], np.float32)

    # bq shifts scores per key position and would change the top-k selection;
    # the device kernel assumes it is zero (it always is from setup_inputs).
    # (bk only shifts each query row uniformly - a softmax no-op. bv and bo
    # are applied exactly on the host below.)
    if np.any(bq):
        return _numpy_reference(x, Wq, bq, Wk, bk, Wv, bv, Wo, bo)

    _install_patches()
    from concourse.bass_utils import run_bass_kernel_spmd

    if "nc" not in _CACHE:
        _CACHE["nc"] = _build_bass()
    nc = _CACHE["nc"]

    WqT = Wq.T.copy()
    WkT = Wk.T.copy()
    WvT = Wv.T.copy()
    WoT = Wo.T.copy()

    in_maps = []
    for c in range(NCORES):
        b = c // 4
        h0 = 2 * (c % 4)
        he = slice(h0 * E, (h0 + 2) * E)
        xT = np.ascontiguousarray(x[b].T)
        xh = _bf16(xT)
        xlo = _bf16(xT - xh.astype(np.float32))
        Aq = WqT[:, he]
        Ak = WkT[:, he]
        aqh = _bf16(Aq)
        aql = _bf16(Aq - aqh.astype(np.float32))
        akh = _bf16(Ak)
        akl = _bf16(Ak - akh.astype(np.float32))
        in_maps.append({
            "xh": np.ascontiguousarray(xh).reshape(4, 128, L),
            "xl": np.ascontiguousarray(xlo).reshape(4, 128, L),
            "aq": np.ascontiguousarray(
                np.stack([aqh, aql])).reshape(2, 4, 128, 128),
            "ak": np.ascontiguousarray(
                np.stack([akh, akl])).reshape(2, 4, 128, 128),
            "av": np.ascontiguousarray(_bf16(WvT[:, he])).reshape(4, 128, 128),
            "wo": np.ascontiguousarray(_bf16(WoT[he, :])),
        })

    res = run_bass_kernel_spmd(nc, in_maps, core_ids=list(range(NCORES)))

    y = np.zeros((B, L, D), np.float32)
    for c in range(NCORES):
        y[c // 4] += res.results[c]["y"]
    y += (bv @ WoT + bo)[None, None, :]
    return y


if __name__ == "__main__":
    rng = np.random.default_rng(0)
    ins = {
        "x": rng.standard_normal((B, L, D), dtype=np.float32),
        "Wq": rng.standard_normal((D, D), dtype=np.float32) * 0.02,
        "bq": np.zeros(D, np.float32),
        "Wk": rng.standard_normal((D, D), dtype=np.float32) * 0.02,
        "bk": np.zeros(D, np.float32),
        "Wv": rng.standard_normal((D, D), dtype=np.float32) * 0.02,
        "bv": np.zeros(D, np.float32),
        "Wo": rng.standard_normal((D, D), dtype=np.float32) * 0.02,
        "bo": np.zeros(D, np.float32),
    }
    out = kernel(**ins)
    ref = _numpy_reference(**ins)
    err = np.linalg.norm(out - ref) / np.linalg.norm(ref)
    print("self-check rel err:", err)
